# revision 23
# baseline (speedup 1.0000x reference)
"""PCEN (per-channel energy normalization) Trainium2 Bass kernel.

Computation (matches the reference nn module):
    m_t = (1-S)*m_{t-1} + S*x_t  along time (last axis), m_{-1} = 0, S = 0.5
    out = (x / (EPS + m)**alpha + delta)**r - delta**r

Strategy: shard the 1024 frequency rows across 8 NeuronCores (128 rows per
core = exactly one SBUF partition dim). Per core, stream ragged time tiles:

    DMA in -> DVE tensor_tensor_scan (EMA; m_t = 0.5*(m_{t-1}+x_t))
    -> ACT Ln(m+eps) -> ACT Exp(-alpha*z - ln(delta))   [p = (eps+m)^-a / d]
    -> DVE mult (t = x*p, in place on the x tile)       [t = u/delta in [0,1)]
    -> ACT Square(a*t + b)                              [quadratic final pow]
    -> out = -S + C  (DVE tensor_scalar 2x for ~30% of tiles, ACT Copy
       with scale=-1/bias=C for the rest -- balances the two engines;
       GPSIMD is NOT used for tensor work: its SBUF port is shared with
       the DVE and measurably stretches the scan/mult when active)
    -> DMA out.

The final (u+delta)^r - delta^r is evaluated as a minimax quadratic in
t = u/delta (t <= 2^alpha/2 < 1 since m >= x/2):
    out ~= c1*t + c2*t^2  =  -(a*t + b)^2 + b^2
with a = sqrt(-c2), b = -c1/(2a). The quadratic is realized by ONE ACT
Square pass (its free input affine provides a*t+b) plus one 2-scalar-op
tensor_scalar / ACT Copy affine (mult -1, add b^2). Q(0)=0 exactly,
Q'(0)~=g'(0), so relative error stays bounded for small outputs;
worst-case rel err ~0.9% incl fp32 rounding (fit computed at build time;
falls back to exact Ln/Exp final pow if the fit exceeds 1e-2).

Why this wins: Ln, Exp and Square live in the SAME activation table set
(natural_log_exp_and_others), so ACT does 3 passes with zero table
reloads (Sqrt would thrash the table set every tile, ~2.7us/reload).
Engine budget per core: DVE scan 70.6us (2 cyc/col, irreducible; modes=[]
so bf16 doesn't help) + fp32 TT mult 35.5 (1x; 2x needs all-16-bit
operands and no cheap conversion engine exists -- GPSIMD tensor ops
measured 2-4.6 cyc/elem AND stretch DVE via the shared SBUF port, so
GPSIMD only does memsets); ACT 3 passes ~92us. The final affine is split
~1/3 DVE tensor_scalar (2x) / ~2/3 ACT Copy to balance both engines at
~112us, against a ~94-109us DMA floor (16MiB in + 16MiB out per core).
Tiles are scan-independent: each is seeded with a HALO-column lookback
(EMA forgets at 0.5/step), so there is no serial carry chain. Each
tile's ACT Copy final (+ its out-DMA) is emitted AFTER the next tile's
ln/exp so the static per-engine order never stalls the DVE mult behind
a finished tile's final.
"""

import numpy as np

S = 0.5
EPS = 1e-6

N_CORES = 8
ROWS = 1024
T_FULL = 32768
RS = ROWS // N_CORES  # 128 rows per core == SBUF partition count


def _fit_quadratic_final(alpha_f, r_f, delta_f):
    """Fit out(t) ~= delta^r*((1+t)^r - 1) on t in [0, 2^alpha/delta] by
    Q(t) = c1*t + c2*t^2 (Q(0)=0), minimizing max |Q-g| / max(|g|, floor).
    Returns (c1, c2, rel_err)."""
    tmax = 2.0 ** float(alpha_f) / float(delta_f)
    dr = float(delta_f) ** float(r_f)
    t = np.linspace(0.0, tmax, 4001)
    g = dr * ((1.0 + t) ** float(r_f) - 1.0)
    gmax = np.abs(g).max()
    w = np.maximum(np.abs(g), 1e-3 * gmax)

    # seed: least squares through the weighted residual
    A = np.stack([t, t * t], axis=1)
    c1, c2 = np.linalg.lstsq(A / w[:, None], g / w, rcond=None)[0]

    def crit(c1, c2):
        e = np.abs(c1 * t + c2 * t * t - g)
        return max(e.max() / gmax, (e / w).max())

    best = crit(c1, c2)
    # local 2D refinement (3 shrinking grids)
    span1, span2 = 0.05, 0.05
    for _ in range(4):
        cc1 = np.linspace(c1 - span1, c1 + span1, 41)
        cc2 = np.linspace(c2 - span2, c2 + span2, 41)
        for a_ in cc1:
            for b_ in cc2:
                c = crit(a_, b_)
                if c < best:
                    best, c1, c2 = c, a_, b_
        span1 *= 0.12
        span2 *= 0.12
    return float(c1), float(c2), float(best)


def _build_and_run(x, alpha_f, r_f, delta_f, trace=False, tmpdir=None):
    import concourse.bacc as bacc
    import concourse.mybir as mybir
    import concourse.tile as tile
    from concourse.bass_utils import run_bass_kernel_spmd

    fp32 = mybir.dt.float32
    Alu = mybir.AluOpType
    Act = mybir.ActivationFunctionType

    delta_r = float(delta_f) ** float(r_f)

    # Quadratic final pow: out = -(a*t+b)^2 + b^2 with t = x*(eps+m)^-alpha/delta
    c1, c2, fit_err = _fit_quadratic_final(alpha_f, r_f, delta_f)
    use_square_final = fit_err < 1e-2 and c2 < 0.0 and c1 > 0.0
    if use_square_final:
        sq_a = float(np.sqrt(-c2))
        sq_b = float(-c1 / (2.0 * sq_a))
        sq_c = float(sq_b * sq_b)

    class _Bacc(bacc.Bacc):
        """Bacc whose activation-table pass prefers sets covering ALL the
        activation functions this kernel uses, so interleaved Ln/Exp/Square
        resolve to one combined table set (natural_log_exp_and_others)
        instead of thrashing between per-function sets (~2.7us per reload)."""

        def insert_act_table_loads(self):
            import bass_rust as _bass_rust
            from concourse.hw_specs import get_activation_tables

            used = {
                i.func
                for b in self.main_func.blocks
                for i in b.instructions
                if isinstance(i, mybir.InstActivation)
            }
            if not used:
                return
            tables = []
            for name, fns in get_activation_tables(self.m.arch).items():
                inter = fns & used
                # If the set doesn't contain ALL used fns, strip the used fns
                # from it so the selector must pick a covering set (index is
                # preserved; ids still match act_info.json).
                if inter and not used.issubset(fns):
                    fns = fns - used
                tables.append((name, fns))
            if not any(used.issubset(fns) for _, fns in tables):
                # No single covering set exists; fall back to default policy.
                tables = list(get_activation_tables(self.m.arch).items())
            _bass_rust.insert_act_table_loads(self, tables)

    nc = _Bacc(
        "TRN2", target_bir_lowering=False, debug=False, num_devices=N_CORES
    )
    x_ap = nc.dram_tensor("x", [RS, T_FULL], fp32, kind="ExternalInput").ap()
    y_ap = nc.dram_tensor("y", [RS, T_FULL], fp32, kind="ExternalOutput").ap()

    # Ragged tiling: small tiles at the start (fast pipeline fill) and at the
    # end (short serial drain chain); big tiles in the middle.
    sizes = [512, 768, 1024, 1536, 2048, 2560, 3072, 4096, 4096, 4096, 4096,
             2048, 1280, 768, 512, 256]
    assert sum(sizes) == T_FULL
    # The final affine (out = -S + C) is split between DVE tensor_scalar
    # (2x) and ACT Copy (1x) to balance the engines:
    #  - tiles 0-1 (ramp, ACT is starved): all ACT
    #  - tiles 2-3 and the trailing small tiles: all DVE
    #  - big tiles (>=2048 cols): first half on DVE, rest on ACT, so
    #    neither engine sees a big blocking final and both halves of a
    #    tile's final run concurrently.

    def final_split(i, size):
        """-> (dve_cols, act_cols) for the final affine of tile i."""
        if i <= 1:
            return 0, size
        if size >= 2048:
            return 640, size - 640
        return size, 0
    # Each tile's scan is seeded by a HALO-column lookback instead of the
    # previous tile's carry: the EMA forgets at 0.5/step, so 32 warmup steps
    # leave < 3e-10 absolute error -- negligible. This makes every tile's
    # scan independent (no serial chain, no carry tiles).
    HALO = 32

    with tile.TileContext(nc) as tc:
        with (
            tc.tile_pool(name="const", bufs=1) as cpool,
            tc.tile_pool(name="xu", bufs=4) as xpool,
            tc.tile_pool(name="m", bufs=7) as mpool,
        ):
            half = cpool.tile([RS, max(sizes) + HALO], fp32)
            # Split the memset so the first (small) scans only wait ~1us for
            # their slice of the 0.5-constant, not the whole fill.
            nc.gpsimd.memset(half[:, : sizes[0]], 1.0 - S)
            nc.gpsimd.memset(half[:, sizes[0] :], 1.0 - S)
            eps_b = cpool.tile([RS, 1], fp32, tag="eps_b")
            nc.gpsimd.memset(eps_b[:], float(EPS))
            # exp bias: -ln(delta) folds the /delta into the exp pass
            lnd_b = cpool.tile([RS, 1], fp32, tag="lnd_b")
            nc.gpsimd.memset(lnd_b[:], float(-np.log(float(delta_f))))
            if use_square_final:
                sqb_b = cpool.tile([RS, 1], fp32, tag="sqb_b")
                nc.gpsimd.memset(sqb_b[:], sq_b)
            else:
                delta_b = cpool.tile([RS, 1], fp32, tag="delta_b")
                nc.gpsimd.memset(delta_b[:], float(delta_f))

            # Deferred emission of ACT Copy finals: tile i's Copy-half (and
            # its out-DMA, which must follow it) enters the program AFTER
            # tile i+1's ln/exp, so the static per-engine order lets ACT
            # keep feeding the DVE mult instead of stalling it behind a
            # finished tile's final.
            pending_act_final = []

            def flush_act_final():
                if pending_act_final:
                    mv_, lo, ysl = pending_act_final.pop(0)
                    nc.scalar.activation(
                        mv_[:, lo:], mv_[:, lo:], Act.Copy,
                        bias=sq_c, scale=-1.0,
                    )
                    nc.sync.dma_start(ysl, mv_)

            start = 0
            for i, size in enumerate(sizes):
                halo = HALO if i > 0 else 0
                n = size + halo  # columns in this tile incl. warmup
                xt = xpool.tile([RS, n], fp32, tag="xu")
                nc.sync.dma_start(xt[:], x_ap[:, start - halo : start + size])

                mt = mpool.tile([RS, n], fp32, tag="m")
                # m_t = (x_t + m_{t-1}) * 0.5  == EMA with S = 0.5
                nc.vector.tensor_tensor_scan(
                    mt[:],
                    xt[:],
                    half[:, :n],
                    initial=0.0,
                    op0=Alu.add,
                    op1=Alu.mult,
                )
                mv = mt[:, halo:n]
                xv = xt[:, halo:n]
                # m <- ln(m + EPS)
                nc.scalar.activation(mv, mv, Act.Ln, bias=eps_b[:])
                if use_square_final:
                    # m <- exp(-alpha*z - ln d) == (EPS+m)^(-alpha) / delta
                    nc.scalar.activation(
                        mv, mv, Act.Exp, scale=-float(alpha_f), bias=lnd_b[:]
                    )
                    # previous tile's deferred Copy final + out-DMA go here,
                    # after this tile's exp in ACT program order
                    flush_act_final()
                    # x <- x * p  (= t = u/delta, in [0, 2^alpha/delta]).
                    # This is the last read of the x tile: writing t (and
                    # everything after) into the m tile lets the xpool slot
                    # recycle immediately, so loads run far ahead of stores.
                    nc.vector.tensor_tensor(xv, xv, mv, Alu.mult)
                    # m <- (a*t + b)^2
                    nc.scalar.activation(
                        mv, xv, Act.Square, scale=sq_a, bias=sqb_b[:]
                    )
                    # m <- -S + b^2   (== c1*t + c2*t^2 ~= out, exact 0 at t=0)
                    dve_cols, act_cols = final_split(i, size)
                    if dve_cols:
                        nc.vector.tensor_scalar(
                            mv[:, :dve_cols], mv[:, :dve_cols],
                            -1.0, sq_c, Alu.mult, Alu.add,
                        )
                    if act_cols:
                        pending_act_final.append(
                            (mv, dve_cols, y_ap[:, start : start + size])
                        )
                    else:
                        nc.sync.dma_start(y_ap[:, start : start + size], mv)
                else:
                    # exact fallback: p = (EPS+m)^(-alpha)
                    nc.scalar.activation(
                        mv, mv, Act.Exp, scale=-float(alpha_f)
                    )
                    nc.vector.tensor_tensor(xv, xv, mv, Alu.mult)
                    nc.scalar.activation(xv, xv, Act.Ln, bias=delta_b[:])
                    nc.scalar.activation(xv, xv, Act.Exp, scale=float(r_f))
                    nc.vector.tensor_scalar(
                        xv, xv, delta_r, None, Alu.subtract
                    )
                    nc.sync.dma_start(y_ap[:, start : start + size], xv)
                start += size
            while pending_act_final:
                flush_act_final()

    nc.compile()

    in_maps = [
        {"x": np.ascontiguousarray(x[c * RS : (c + 1) * RS])}
        for c in range(N_CORES)
    ]
    res = run_bass_kernel_spmd(
        nc, in_maps, list(range(N_CORES)), trace=trace, tmpdir=tmpdir
    )
    out = np.concatenate(
        [res.results[c]["y"] for c in range(N_CORES)], axis=0
    ).astype(np.float32)
    return out, res


def kernel(x, alpha, r, delta):
    x = np.asarray(x, dtype=np.float32)
    assert x.shape == (ROWS, T_FULL), x.shape
    out, _ = _build_and_run(x, float(alpha), float(r), float(delta))
    return out


# revision 27
# speedup vs baseline: 1.0103x; 1.0103x over previous
"""PCEN (per-channel energy normalization) Trainium2 Bass kernel.

Computation (matches the reference nn module):
    m_t = (1-S)*m_{t-1} + S*x_t  along time (last axis), m_{-1} = 0, S = 0.5
    out = (x / (EPS + m)**alpha + delta)**r - delta**r

Strategy: shard the 1024 frequency rows across 8 NeuronCores (128 rows per
core = exactly one SBUF partition dim). Per core, stream ragged time tiles:

    DMA in -> DVE tensor_tensor_scan (EMA; m_t = 0.5*(m_{t-1}+x_t))
    -> ACT Ln(m+eps) -> ACT Exp(-alpha*z - ln(delta))   [p = (eps+m)^-a / d]
    -> DVE mult (t = x*p, in place on the x tile)       [t = u/delta in [0,1)]
    -> ACT Square(a*t + b)                              [quadratic final pow]
    -> out = -S + C  (DVE tensor_scalar 2x for ~30% of tiles, ACT Copy
       with scale=-1/bias=C for the rest -- balances the two engines;
       GPSIMD is NOT used for tensor work: its SBUF port is shared with
       the DVE and measurably stretches the scan/mult when active)
    -> DMA out.

The final (u+delta)^r - delta^r is evaluated as a minimax quadratic in
t = u/delta (t <= 2^alpha/2 < 1 since m >= x/2):
    out ~= c1*t + c2*t^2  =  -(a*t + b)^2 + b^2
with a = sqrt(-c2), b = -c1/(2a). The quadratic is realized by ONE ACT
Square pass (its free input affine provides a*t+b) plus one 2-scalar-op
tensor_scalar / ACT Copy affine (mult -1, add b^2). Q(0)=0 exactly,
Q'(0)~=g'(0), so relative error stays bounded for small outputs;
worst-case rel err ~0.9% incl fp32 rounding (fit computed at build time;
falls back to exact Ln/Exp final pow if the fit exceeds 1e-2).

Why this wins: Ln, Exp and Square live in the SAME activation table set
(natural_log_exp_and_others), so ACT does 3 passes with zero table
reloads (Sqrt would thrash the table set every tile, ~2.7us/reload).
Engine budget per core: DVE scan 70.6us (2 cyc/col, irreducible; modes=[]
so bf16 doesn't help) + fp32 TT mult 35.5 (1x; 2x needs all-16-bit
operands and no cheap conversion engine exists -- GPSIMD tensor ops
measured 2-4.6 cyc/elem AND stretch DVE via the shared SBUF port, so
GPSIMD only does memsets); ACT 3 passes ~92us. The final affine is split
~1/3 DVE tensor_scalar (2x) / ~2/3 ACT Copy to balance both engines at
~112us, against a ~94-109us DMA floor (16MiB in + 16MiB out per core).
Tiles are scan-independent: each is seeded with a HALO-column lookback
(EMA forgets at 0.5/step), so there is no serial carry chain. Each
tile's ACT Copy final (+ its out-DMA) is emitted AFTER the next tile's
ln/exp so the static per-engine order never stalls the DVE mult behind
a finished tile's final.
"""

import numpy as np

S = 0.5
EPS = 1e-6

N_CORES = 8
ROWS = 1024
T_FULL = 32768
RS = ROWS // N_CORES  # 128 rows per core == SBUF partition count


def _fit_quadratic_final(alpha_f, r_f, delta_f):
    """Fit out(t) ~= delta^r*((1+t)^r - 1) on t in [0, 2^alpha/delta] by
    Q(t) = c1*t + c2*t^2 (Q(0)=0), minimizing max |Q-g| / max(|g|, floor).
    Returns (c1, c2, rel_err)."""
    tmax = 2.0 ** float(alpha_f) / float(delta_f)
    dr = float(delta_f) ** float(r_f)
    t = np.linspace(0.0, tmax, 4001)
    g = dr * ((1.0 + t) ** float(r_f) - 1.0)
    gmax = np.abs(g).max()
    w = np.maximum(np.abs(g), 1e-3 * gmax)

    # seed: least squares through the weighted residual
    A = np.stack([t, t * t], axis=1)
    c1, c2 = np.linalg.lstsq(A / w[:, None], g / w, rcond=None)[0]

    def crit(c1, c2):
        e = np.abs(c1 * t + c2 * t * t - g)
        return max(e.max() / gmax, (e / w).max())

    best = crit(c1, c2)
    # local 2D refinement (3 shrinking grids)
    span1, span2 = 0.05, 0.05
    for _ in range(4):
        cc1 = np.linspace(c1 - span1, c1 + span1, 41)
        cc2 = np.linspace(c2 - span2, c2 + span2, 41)
        for a_ in cc1:
            for b_ in cc2:
                c = crit(a_, b_)
                if c < best:
                    best, c1, c2 = c, a_, b_
        span1 *= 0.12
        span2 *= 0.12
    return float(c1), float(c2), float(best)


def _build_and_run(x, alpha_f, r_f, delta_f, trace=False, tmpdir=None):
    import concourse.bacc as bacc
    import concourse.mybir as mybir
    import concourse.tile as tile
    from concourse.bass_utils import run_bass_kernel_spmd

    fp32 = mybir.dt.float32
    Alu = mybir.AluOpType
    Act = mybir.ActivationFunctionType

    delta_r = float(delta_f) ** float(r_f)

    # Quadratic final pow: out = -(a*t+b)^2 + b^2 with t = x*(eps+m)^-alpha/delta
    c1, c2, fit_err = _fit_quadratic_final(alpha_f, r_f, delta_f)
    use_square_final = fit_err < 1e-2 and c2 < 0.0 and c1 > 0.0
    if use_square_final:
        sq_a = float(np.sqrt(-c2))
        sq_b = float(-c1 / (2.0 * sq_a))
        sq_c = float(sq_b * sq_b)

    class _Bacc(bacc.Bacc):
        """Bacc whose activation-table pass prefers sets covering ALL the
        activation functions this kernel uses, so interleaved Ln/Exp/Square
        resolve to one combined table set (natural_log_exp_and_others)
        instead of thrashing between per-function sets (~2.7us per reload)."""

        def insert_act_table_loads(self):
            import bass_rust as _bass_rust
            from concourse.hw_specs import get_activation_tables

            used = {
                i.func
                for b in self.main_func.blocks
                for i in b.instructions
                if isinstance(i, mybir.InstActivation)
            }
            if not used:
                return
            tables = []
            for name, fns in get_activation_tables(self.m.arch).items():
                inter = fns & used
                # If the set doesn't contain ALL used fns, strip the used fns
                # from it so the selector must pick a covering set (index is
                # preserved; ids still match act_info.json).
                if inter and not used.issubset(fns):
                    fns = fns - used
                tables.append((name, fns))
            if not any(used.issubset(fns) for _, fns in tables):
                # No single covering set exists; fall back to default policy.
                tables = list(get_activation_tables(self.m.arch).items())
            _bass_rust.insert_act_table_loads(self, tables)

    nc = _Bacc(
        "TRN2", target_bir_lowering=False, debug=False, num_devices=N_CORES
    )
    x_ap = nc.dram_tensor("x", [RS, T_FULL], fp32, kind="ExternalInput").ap()
    y_ap = nc.dram_tensor("y", [RS, T_FULL], fp32, kind="ExternalOutput").ap()

    # Ragged tiling: small tiles at the start (fast pipeline fill) and at the
    # end (short serial drain chain); big tiles in the middle.
    sizes = [512, 768, 1024, 1536, 2048, 2560, 3072, 4096, 4096, 4096, 4096,
             2048, 1280, 768, 512, 256]
    assert sum(sizes) == T_FULL
    # The final affine (out = -S + C) is split between DVE tensor_scalar
    # (2x) and ACT Copy (1x) to balance the engines:
    #  - tiles 0-1 (ramp, ACT is starved): all ACT
    #  - tiles 2-3 and the trailing small tiles: all DVE
    #  - big tiles (>=2048 cols): first half on DVE, rest on ACT, so
    #    neither engine sees a big blocking final and both halves of a
    #    tile's final run concurrently.

    def final_split(i, size):
        """-> (dve_cols, act_cols) for the final affine of tile i."""
        if i <= 1:
            return 0, size
        if size >= 2048:
            return 640, size - 640
        return size, 0
    # Each tile's scan is seeded by a HALO-column lookback instead of the
    # previous tile's carry: the EMA forgets at 0.5/step, so 32 warmup steps
    # leave < 3e-10 absolute error -- negligible. This makes every tile's
    # scan independent (no serial chain, no carry tiles).
    HALO = 32

    with tile.TileContext(nc) as tc:
        with (
            tc.tile_pool(name="const", bufs=1) as cpool,
            tc.tile_pool(name="xu", bufs=6) as xpool,
            tc.tile_pool(name="m", bufs=5) as mpool,
        ):
            half = cpool.tile([RS, max(sizes) + HALO], fp32)
            # Split the memset so the first (small) scans only wait ~1us for
            # their slice of the 0.5-constant, not the whole fill.
            nc.gpsimd.memset(half[:, : sizes[0]], 1.0 - S)
            nc.gpsimd.memset(half[:, sizes[0] :], 1.0 - S)
            eps_b = cpool.tile([RS, 1], fp32, tag="eps_b")
            nc.gpsimd.memset(eps_b[:], float(EPS))
            # Dummy 1-col activation issued at program start: hoists the
            # ~1.3us ACT_TABLE_LOAD into the boot phase, off the first
            # tile's critical path.
            warm = cpool.tile([RS, 1], fp32, tag="warm")
            nc.gpsimd.memset(warm[:], 1.0)
            nc.scalar.activation(warm[:], warm[:], Act.Ln, bias=eps_b[:])
            # exp bias: -ln(delta) folds the /delta into the exp pass
            lnd_b = cpool.tile([RS, 1], fp32, tag="lnd_b")
            nc.gpsimd.memset(lnd_b[:], float(-np.log(float(delta_f))))
            if use_square_final:
                sqb_b = cpool.tile([RS, 1], fp32, tag="sqb_b")
                nc.gpsimd.memset(sqb_b[:], sq_b)
            else:
                delta_b = cpool.tile([RS, 1], fp32, tag="delta_b")
                nc.gpsimd.memset(delta_b[:], float(delta_f))

            # Deferred emission of ACT Copy finals: tile i's Copy-half (and
            # its out-DMA, which must follow it) enters the program AFTER
            # tile i+1's ln/exp, so the static per-engine order lets ACT
            # keep feeding the DVE mult instead of stalling it behind a
            # finished tile's final.
            pending_act_final = []

            def flush_act_final():
                if pending_act_final:
                    xv_, mv_, lo, ysl = pending_act_final.pop(0)
                    nc.scalar.activation(
                        xv_[:, lo:], mv_[:, lo:], Act.Copy,
                        bias=sq_c, scale=-1.0,
                    )
                    nc.sync.dma_start(ysl, xv_)

            start = 0
            for i, size in enumerate(sizes):
                halo = HALO if i > 0 else 0
                n = size + halo  # columns in this tile incl. warmup
                xt = xpool.tile([RS, n], fp32, tag="xu")
                nc.sync.dma_start(xt[:], x_ap[:, start - halo : start + size])

                mt = mpool.tile([RS, n], fp32, tag="m")
                # m_t = (x_t + m_{t-1}) * 0.5  == EMA with S = 0.5
                nc.vector.tensor_tensor_scan(
                    mt[:],
                    xt[:],
                    half[:, :n],
                    initial=0.0,
                    op0=Alu.add,
                    op1=Alu.mult,
                )
                mv = mt[:, halo:n]
                xv = xt[:, halo:n]
                # m <- ln(m + EPS)
                nc.scalar.activation(mv, mv, Act.Ln, bias=eps_b[:])
                if use_square_final:
                    # m <- exp(-alpha*z - ln d) == (EPS+m)^(-alpha) / delta
                    nc.scalar.activation(
                        mv, mv, Act.Exp, scale=-float(alpha_f), bias=lnd_b[:]
                    )
                    # previous tile's deferred Copy final + out-DMA go here,
                    # after this tile's exp in ACT program order
                    flush_act_final()
                    # x <- x * p  (= t = u/delta, in [0, 2^alpha/delta])
                    nc.vector.tensor_tensor(xv, xv, mv, Alu.mult)
                    # m <- (a*t + b)^2
                    nc.scalar.activation(
                        mv, xv, Act.Square, scale=sq_a, bias=sqb_b[:]
                    )
                    # x <- -S + b^2   (== c1*t + c2*t^2 ~= out, exact 0 at t=0)
                    dve_cols, act_cols = final_split(i, size)
                    if dve_cols:
                        nc.vector.tensor_scalar(
                            xv[:, :dve_cols], mv[:, :dve_cols],
                            -1.0, sq_c, Alu.mult, Alu.add,
                        )
                    if act_cols:
                        pending_act_final.append(
                            (xv, mv, dve_cols, y_ap[:, start : start + size])
                        )
                    else:
                        nc.sync.dma_start(y_ap[:, start : start + size], xv)
                else:
                    # exact fallback: p = (EPS+m)^(-alpha)
                    nc.scalar.activation(
                        mv, mv, Act.Exp, scale=-float(alpha_f)
                    )
                    nc.vector.tensor_tensor(xv, xv, mv, Alu.mult)
                    nc.scalar.activation(xv, xv, Act.Ln, bias=delta_b[:])
                    nc.scalar.activation(xv, xv, Act.Exp, scale=float(r_f))
                    nc.vector.tensor_scalar(
                        xv, xv, delta_r, None, Alu.subtract
                    )
                    nc.sync.dma_start(y_ap[:, start : start + size], xv)
                start += size
            while pending_act_final:
                flush_act_final()

    nc.compile()

    in_maps = [
        {"x": np.ascontiguousarray(x[c * RS : (c + 1) * RS])}
        for c in range(N_CORES)
    ]
    res = run_bass_kernel_spmd(
        nc, in_maps, list(range(N_CORES)), trace=trace, tmpdir=tmpdir
    )
    out = np.concatenate(
        [res.results[c]["y"] for c in range(N_CORES)], axis=0
    ).astype(np.float32)
    return out, res


def kernel(x, alpha, r, delta):
    x = np.asarray(x, dtype=np.float32)
    assert x.shape == (ROWS, T_FULL), x.shape
    out, _ = _build_and_run(x, float(alpha), float(r), float(delta))
    return out
